# revision 36
# baseline (speedup 1.0000x reference)
"""Block-local sparse attention (LSG-style) on 8 TRN2 NeuronCores.

Sharding: the 32 (n, h) pairs are split 4-per-core (data/head parallel, no
collectives). Host-side numpy prep re-lays-out the inputs so the device
kernel needs no transposes, all bf16:

  - qt : Q^T per head, zero-padded to [128, T]
  - lkt/skt/gkt: local/sparse/global K^T, token-padded with zeros and
    row-padded to 128 partitions.  Every matmul in the kernel then uses the
    same 128-row PE tile shape; the padded stationary rows are zero so the
    junk moving rows contribute nothing.  Uniform tile shapes let the PE
    preload every stationary during the previous matmul (no reload stall at
    64<->128 switches) and keep the PE continuously busy, which lets the
    hardware ramp the tensor-engine clock from 1.2 to 2.4 GHz.
  - lv/sv/gv: V with a ones column appended (col 64), chunked [128, c, 65],
    every row scaled by exp(mask): softmax(QK/8 + m) @ V is computed as
    sum_t exp(s_t) e^{m_t} [V_t, 1] followed by a divide by the accumulated
    last column, so no mask row and no max-subtraction are needed.
    sv holds 4 phase-shifted copies so the 32-token-granular sparse windows
    always start at partition 0 (stored chunk-major so chunk-range loads
    are one descriptor per partition).

The device processes query-block PAIRS: 9 score matmuls per pair (the
shared global chunk and the two shared local chunks stream both blocks'
256 q columns in one matmul) into a 3-bank PSUM region [128, 1536], one
exp(S/8) on ACT into bf16 pp, then 12 PV matmuls (6 per block, N=65)
accumulating [q, V|Z], and a reciprocal-normalize on DVE.  ACT (~1.3us per
pair) and the PE (~1.0us at full clock) are the joint bottlenecks.

Pipelining:
  - All loads and stores share the single qSPDynamicHW DGE, which executes
    DMA instructions serially, so big input batches delay store
    completions.  Inputs load in 3-4 phases per slot (column prefixes
    covering pairs 0-3 / 4-9 / 10-15), issued between stores of earlier
    slots so no burst exceeds ~2us; compute starts after ~0.5 MB.
  - Phase-completion gates use two alternating semaphores per slot parity,
    and each phase's first DMA waits on the previous same-semaphore
    phase's total: DMA completions are unordered across instructions, so a
    gate value is race-free only if no later increment can cross it.
  - Output of a pair is ONE dma (dram laid out [SL, B, NB, D], transposed
    back on the host); ob/rec are quad-buffered with four store semaphores
    so a store delayed behind an input batch has 8 pair-periods of slack.
  - pp is triple-buffered so exp(p) only waits on PV(p-3).

Known hardware landmine (cost several device wedges): an engine reading a
PSUM bank while another engine concurrently touches the same bank (ACT
read + DVE read, or PE matmul write + DVE read) hard-crashes the device
(NRT_EXEC_UNIT_UNRECOVERABLE).  Offloading part of the exp to the DVE via
the Schraudolph int16/bf16 bit trick works numerically (7.7e-3 end to
end) but requires a PSUM bank partitioning between ACT and DVE that did
not fit the 8-bank budget in time; the DVE therefore only touches the pv
banks (strict parity alternation against the PE).
"""

from contextlib import ExitStack

import numpy as np

import concourse.bass as bass
import concourse.mybir as mybir
from concourse.bass_utils import run_bass_kernel_spmd

N, H, T, D = 2, 16, 4096, 64
B = 128          # query block
NB = T // B      # 32
G = 64           # global tokens
TSP = T // 4     # sparse tokens (1024)
NH = N * H       # 32
NCORES = 8
SL = NH // NCORES  # 4 heads per core
NP = SL * NB // 2  # 64 block-pairs per core
PPS = NB // 2      # 16 pairs per slot

LKT_W = T + 2 * B            # 4352 padded local tokens
SKT_W = TSP + 320            # 1344 padded sparse tokens
LV_C = LKT_W // 128          # 34 local V chunks
SV_C = 11                    # sparse V chunks per phase

F32 = mybir.dt.float32
BF16 = mybir.dt.bfloat16
GE = "sem-ge"

# column layout of the per-pair score/prob tile [128, 1536] (3 PSUM banks;
# regions never cross a 512-col bank boundary).  Cols 0:768 = sparse+global
# (ACT exp part A), 768:1280 = shared local chunks (ACT exp part B),
# 1280:1536 = single-block local chunks, exp'd on DVE via the Schraudolph
# int16/bf16 bit trick (rel err ~2%, verified 7.7e-3 end to end).
C_SP1A, C_SP1B = 0, 128
C_SP2A, C_SP2B = 256, 384
C_G = 512        # 256 wide: q of both blocks
C_LOC1 = 768     # 256 wide: local chunk b+1, both blocks
C_LOC2 = 1024    # 256 wide: local chunk b+2, both blocks
C_LOC0 = 1280    # 128: local chunk b, block A only (DVE exp)
C_LOC3 = 1408    # 128: local chunk b+3, block B only (DVE exp)

# Schraudolph constants: bf16 bits of exp(x*0.125) ~= int16(x*SCH_A + SCH_B)
SCH_A = float(128 * 1.4426950408889634 * 0.125)
SCH_B = 16256.0 - 0.057 * 128.0

# per-slot load phases: each phase covers the column needs of a pair range
# (from the access patterns:
#   scores pair p: qt < 256(p+1), lkt < 256p+512, skt < 64p+384
#   PV pair p: lv chunks <= 2p+5, sv chunks <= p//2+2)
# phase = (qt-range, lkt-range, skt-range, lv-chunk-range, sv-chunk-range,
#          gate_hb)  — gate_hb is the first pair needing it.
# Slot 0 uses 4 finer phases so compute starts after ~0.45 MB.
PHASES_S0 = (
    ((0, 512), (0, 768), (0, 448), (0, 8), (0, 3), 0),
    ((512, 1024), (768, 1280), (448, 576), (8, 12), (3, 4), 2),
    ((1024, 2560), (1280, 2816), (576, 960), (12, 24), (4, 7), 4),
    ((2560, 4096), (2816, LKT_W), (960, SKT_W), (24, LV_C), (7, SV_C), 10),
)
PHASES = (
    ((0, 1024), (0, 1280), (0, 576), (0, 12), (0, 5), 0),
    ((1024, 2560), (1280, 2816), (576, 960), (12, 24), (5, 7), 4),
    ((2560, 4096), (2816, LKT_W), (960, SKT_W), (24, LV_C), (7, SV_C), 10),
)
def _slot_phases(s):
    return PHASES_S0 if s == 0 else PHASES


def _phase_ndma(ph):
    # qt, lkt, skt, lv are one DMA each; sv is one DMA per chunk
    return 4 + (ph[4][1] - ph[4][0])


# A semaphore wait is only race-free at the end of a maximal run of
# consecutive instructions updating that semaphore, so consecutive phases
# alternate between two semaphores per slot parity: di[parity][phase_seq%2].
# DI_SEM[(s, k)] = (parity, alt) and DI_GATE[(s, hb)] = (parity, alt, value).
DI_SEM = {}
DI_GATE = {}
for _u in range(2):
    _cum = [0, 0]
    _seq = 0
    for _s in range(_u, SL, 2):
        for _k, _ph in enumerate(_slot_phases(_s)):
            _alt = _seq % 2
            _cum[_alt] += 16 * _phase_ndma(_ph)
            DI_SEM[(_s, _k)] = (_u, _alt)
            DI_GATE[(_s, _ph[5])] = (_u, _alt, _cum[_alt])
            _seq += 1


def _build_bass():
    nc = bass.Bass("TRN2", num_devices=NCORES, debug=False)

    qt = nc.dram_tensor("qt", [SL, 128, T], BF16, kind="ExternalInput")
    lkt = nc.dram_tensor("lkt", [SL, 128, LKT_W], BF16, kind="ExternalInput")
    skt = nc.dram_tensor("skt", [SL, 128, SKT_W], BF16, kind="ExternalInput")
    gkt = nc.dram_tensor("gkt", [128, SL * 128], BF16, kind="ExternalInput")
    lv = nc.dram_tensor("lv", [SL, 128, LV_C * 65], BF16, kind="ExternalInput")
    sv = nc.dram_tensor("sv", [SL, 128, SV_C, 4, 65], BF16, kind="ExternalInput")
    gv = nc.dram_tensor("gv", [128, SL * 65], BF16, kind="ExternalInput")
    # output laid out [SL, B, NB, D] so one DMA covers a pair ([128, 2, 64]);
    # host transposes back to [SL, T, D]
    o = nc.dram_tensor("o", [SL, B, NB, D], F32, kind="ExternalOutput")

    EXP = mybir.ActivationFunctionType.Exp

    with ExitStack() as es:
        ec = es.enter_context
        # double-buffered inputs (slot parity)
        qt_t = [ec(nc.sbuf_tensor(f"qt_t{i}", [128, T], BF16)) for i in range(2)]
        lkt_t = [ec(nc.sbuf_tensor(f"lkt_t{i}", [128, LKT_W], BF16)) for i in range(2)]
        skt_t = [ec(nc.sbuf_tensor(f"skt_t{i}", [128, SKT_W], BF16)) for i in range(2)]
        lv_t = [ec(nc.sbuf_tensor(f"lv_t{i}", [128, LV_C * 65], BF16)) for i in range(2)]
        sv_t = [ec(nc.sbuf_tensor(f"sv_t{i}", [128, SV_C, 4, 65], BF16)) for i in range(2)]
        # globals are tiny: all slots resident, loaded once with one DMA each
        gkt_t = ec(nc.sbuf_tensor("gkt_t", [128, SL * 128], BF16))
        gv_t = ec(nc.sbuf_tensor("gv_t", [128, SL * 65], BF16))
        # per-pair working set
        psS = [ec(nc.psum_tensor(f"psS{i}", [128, 1536], F32)) for i in range(2)]  # 3 banks
        pv = [ec(nc.psum_tensor(f"pv{i}", [128, 512], F32)) for i in range(2)]     # 1 bank
        pp = [ec(nc.sbuf_tensor(f"pp{i}", [128, 1536], BF16)) for i in range(3)]
        rec = [ec(nc.sbuf_tensor(f"rec{i}", [128, 2], F32)) for i in range(4)]
        ob = [ec(nc.sbuf_tensor(f"ob{i}", [128, 128], F32)) for i in range(4)]

        di = [[ec(nc.semaphore(f"di{i}{a}")) for a in range(2)] for i in range(2)]  # input loads, (parity, alternation)
        dg = ec(nc.semaphore("dg"))      # global k/v loads
        st = [ec(nc.semaphore(f"st{i}")) for i in range(4)]  # out stores, p%4 (matches ob buffers)
        pe_s = ec(nc.semaphore("pe_s"))  # +3 per pair: score thirds done
        pe_v = ec(nc.semaphore("pe_v"))  # +1 per pair: PV matmuls done
        act = ec(nc.semaphore("act"))    # +2 per pair: ACT exp halves done
        sch = ec(nc.semaphore("sch"))    # +1 per pair: DVE exp done
        dve = ec(nc.semaphore("dve"))    # +1 per pair: normalize done
        block = ec(nc.Block())

        # last waited-on cumulative value per di semaphore: a later phase
        # crossing that value must itself wait on it (race-checker rule),
        # which is free since the previous same-sem phase finished long ago
        chain = {}

        def phase_pieces(sync, s, k, wait=None):
            u = s % 2
            _, alt = DI_SEM[(s, k)]
            (q0, q1), (l0, l1), (s0, s1), (v0, v1), (c0, c1), _ = _slot_phases(s)[k]
            if wait is not None:
                sync.wait_ge(pe_v, wait)
            dmas = [
                (qt_t[u][:, q0:q1], qt[s, :, q0:q1]),
                (lkt_t[u][:, l0:l1], lkt[s, :, l0:l1]),
                (skt_t[u][:, s0:s1], skt[s, :, s0:s1]),
                (lv_t[u][:, v0 * 65 : v1 * 65], lv[s, :, v0 * 65 : v1 * 65]),
            ] + [
                (sv_t[u][:, c, :, :], sv[s, :, c, :, :]) for c in range(c0, c1)
            ]
            prev = chain.get((u, alt))
            if prev is not None:
                sync.wait_ge(di[u][alt], prev)
            for dst, src in dmas:
                sync.dma_start(dst, src).then_inc(di[u][alt], 16)
            chain[(u, alt)] = DI_GATE[(s, _slot_phases(s)[k][5])][2]

        @block.sync
        def _(sync):
            phase_pieces(sync, 0, 0)
            sync.dma_start(gkt_t[:], gkt[:]).then_inc(dg, 16)
            sync.dma_start(gv_t[:], gv[:]).then_inc(dg, 16)
            for k in range(1, 4):
                phase_pieces(sync, 0, k)
            for p in range(NP):
                s, hb = divmod(p, PPS)
                b = 2 * hb
                u = p % 4
                sync.dma_start(
                    o[s, :, b : b + 2, :], ob[u][:, 0:128]
                ).wait_op(dve, p + 1, GE).then_inc(st[u], 16)
                if s == 0 and hb in (2, 5, 7):
                    # slot 1's phases, spread through slot 0 (buffers fresh)
                    phase_pieces(sync, 1, {2: 0, 5: 1, 7: 2}[hb])
                if hb == 0 and s >= 1 and s + 1 < SL:
                    phase_pieces(sync, s + 1, 2, wait=16 * s)
                if hb == 9 and s + 2 < SL:
                    phase_pieces(sync, s + 2, 0, wait=16 * s + 10)
                if hb == 13 and s + 2 < SL:
                    phase_pieces(sync, s + 2, 1, wait=16 * s + 14)
            for i in range(4):
                sync.wait_ge(st[i], 16 * (NP // 4))

        def emit_scores(p):
            s, hb = divmod(p, PPS)
            b = 2 * hb
            u = p % 2
            su = s % 2
            if (s, hb) in DI_GATE:
                if p == 0:
                    nc.tensor.wait_ge(dg, 32)
                gu, galt, gval = DI_GATE[(s, hb)]
                nc.tensor.wait_ge(di[gu][galt], gval)
            qA = qt_t[su][:, b * B : (b + 1) * B]
            qB = qt_t[su][:, (b + 1) * B : (b + 2) * B]
            qAB = qt_t[su][:, b * B : (b + 2) * B]
            w1a, w2a = 32 * b, 32 * b + 224
            w1b, w2b = w1a + 32, w2a + 32
            mms = (
                (C_SP1A, 128, skt_t[su][:, w1a : w1a + 128], qA),
                (C_SP1B, 128, skt_t[su][:, w1b : w1b + 128], qB),
                (C_SP2A, 128, skt_t[su][:, w2a : w2a + 128], qA),
                (C_SP2B, 128, skt_t[su][:, w2b : w2b + 128], qB),
                (C_G, 256, gkt_t[:, s * 128 : (s + 1) * 128], qAB),
                (C_LOC1, 256, lkt_t[su][:, (b + 1) * B : (b + 2) * B], qAB),
                (C_LOC2, 256, lkt_t[su][:, (b + 2) * B : (b + 3) * B], qAB),
                (C_LOC0, 128, lkt_t[su][:, b * B : (b + 1) * B], qA),
                (C_LOC3, 128, lkt_t[su][:, (b + 3) * B : (b + 4) * B], qB),
            )
            for kk, (col, w, lhsT, rhs) in enumerate(mms):
                mm = nc.tensor.matmul(
                    psS[u][:, col : col + w],
                    lhsT, rhs,
                    start=True, stop=True,
                )
                if kk in (4, 6, 8):
                    mm.then_inc(pe_s, 1)

        def emit_pv(p):
            s, hb = divmod(p, PPS)
            b = 2 * hb
            u = p % 2
            su = s % 2
            j3 = p % 3
            if p >= 2:
                nc.tensor.wait_ge(dve, p - 1)  # pv[u] free
            kk = 0
            for blk in range(2):
                bb = b + blk
                w1, w2 = 32 * bb, 32 * bb + 224
                c1, r1 = divmod(w1, 128)
                c2, r2 = divmod(w2, 128)
                p1, p2 = r1 // 32, r2 // 32
                if blk == 0:
                    # single-block chunk (C_LOC0, DVE-exp'd) last so one
                    # sch wait covers both blocks by program order
                    lhs = (C_SP1A, C_SP2A, C_G, C_LOC1, C_LOC2, C_LOC0)
                    lvs = (bb + 1, bb + 2, bb)
                else:
                    lhs = (C_SP1B, C_SP2B, C_G + 128, C_LOC1 + 128,
                           C_LOC2 + 128, C_LOC3)
                    lvs = (bb, bb + 1, bb + 2)
                rhss = (
                    sv_t[su][:, c1, p1, :],
                    sv_t[su][:, c2, p2, :],
                    gv_t[:, s * 65 : (s + 1) * 65],
                    lv_t[su][:, lvs[0] * 65 : lvs[0] * 65 + 65],
                    lv_t[su][:, lvs[1] * 65 : lvs[1] * 65 + 65],
                    lv_t[su][:, lvs[2] * 65 : lvs[2] * 65 + 65],
                )
                out = pv[u][:, blk * 128 : blk * 128 + 65]
                for j in range(6):
                    mm = nc.tensor.matmul(
                        out, pp[j3][:, lhs[j] : lhs[j] + 128], rhss[j],
                        start=(j == 0), stop=(j == 5),
                    )
                    if kk == 0:
                        mm.wait_op(act, p + 1, GE)  # pp ready
                    if kk == 11:
                        mm.then_inc(pe_v, 1)
                    kk += 1

        @block.tensor
        def _(tensor):
            emit_scores(0)
            emit_scores(1)
            for p in range(NP):
                emit_pv(p)
                if p + 2 < NP:
                    emit_scores(p + 2)

        @block.scalar
        def _(scalar):
            for p in range(NP):
                u = p % 2
                j3 = p % 3
                if p >= 3:
                    scalar.wait_ge(pe_v, p - 2)  # pp[j3] free: PV of p-3 done
                nc.scalar.activation(
                    pp[j3][:], psS[u][:, 0:1536], EXP, scale=0.125
                ).wait_op(pe_s, 3 * p + 3, GE).then_inc(act, 1)

        def emit_sch(p):
            # exp via Schraudolph: bf16 bits of exp(x/8) ~= int16(x*A + B);
            # pp[p%3] was last read by PV(p-3), and the preceding DVE
            # reciprocal already waited pe_v >= p, so no extra WAR wait.
            nc.vector.tensor_scalar(
                pp[p % 3][:, 1280:1536].bitcast(mybir.dt.int16),
                psS[p % 2][:, 1280:1536],
                SCH_A, SCH_B,
                mybir.AluOpType.mult, mybir.AluOpType.add,
            ).wait_op(pe_s, 3 * p + 3, GE).then_inc(sch, 1)

        @block.vector
        def _(vector):
            for p in range(NP):
                u = p % 4
                if p >= 4:
                    # ob[u]/rec[u] free once store(p-4) completed
                    vector.wait_ge(st[u], 16 * ((p - 4) // 4 + 1))
                nc.vector.reciprocal(rec[u][:, 0:1], pv[p % 2][:, 64:65]).wait_op(
                    pe_v, p + 1, GE
                )
                nc.vector.reciprocal(rec[u][:, 1:2], pv[p % 2][:, 192:193])
                nc.vector.drain()
                nc.vector.tensor_mul(
                    ob[u][:, 0:64], pv[p % 2][:, 0:64],
                    rec[u][:, 0:1].broadcast_to([128, 64]),
                )
                nc.vector.tensor_mul(
                    ob[u][:, 64:128], pv[p % 2][:, 128:192],
                    rec[u][:, 1:2].broadcast_to([128, 64]),
                ).then_inc(dve, 1)

    return nc


def _prepare(inputs):
    import ml_dtypes

    bf = ml_dtypes.bfloat16
    f = np.float32
    q = np.asarray(inputs["query_layer"], f).reshape(NH, T, D)
    k = np.asarray(inputs["key_layer"], f).reshape(NH, T, D)
    v = np.asarray(inputs["value_layer"], f).reshape(NH, T, D)
    sk = np.asarray(inputs["sparse_key"], f).reshape(NH, TSP, D)
    svv = np.asarray(inputs["sparse_value"], f).reshape(NH, TSP, D)
    gk = np.asarray(inputs["global_key"], f).reshape(NH, G, D)
    gvv = np.asarray(inputs["global_value"], f).reshape(NH, G, D)
    am = np.repeat(np.asarray(inputs["attention_mask"], f)[:, 0, 0, :], H, 0)
    sm = np.repeat(np.asarray(inputs["sparse_mask"], f)[:, 0, 0, :], H, 0)
    gm = np.repeat(np.asarray(inputs["global_mask"], f)[:, 0, 0, :], H, 0)

    # all K^T stationaries (and the moving q) are padded to 128 rows with
    # zeros so every matmul uses the same 128-row PE tile shape — avoiding
    # the stationary-reload serialization at 64<->128 shape switches
    qt = np.zeros((NH, 128, T), f)
    qt[:, :64] = q.transpose(0, 2, 1)
    qt = qt.astype(bf)

    lkt = np.zeros((NH, 128, LKT_W), f)
    lkt[:, :64, B : B + T] = k.transpose(0, 2, 1)
    lkt = lkt.astype(bf)

    skt = np.zeros((NH, 128, SKT_W), f)
    skt[:, :64, 160 : 160 + TSP] = sk.transpose(0, 2, 1)
    skt = skt.astype(bf)

    gkt = np.zeros((NH, 128, 128), f)
    gkt[:, :64, :G] = gk.transpose(0, 2, 1)
    gkt = gkt.astype(bf)
    # per-core: [128, SL*128] (slot-minor so one DMA loads all slots)
    gktj = np.ascontiguousarray(
        gkt.reshape(NCORES, SL, 128, 128).transpose(0, 2, 1, 3)
    ).reshape(NCORES, 128, SL * 128)

    # V_aug rows scaled by exp(mask); pad rows are all-zero
    em_l = np.zeros((NH, LKT_W), f)
    em_l[:, B : B + T] = np.exp(am)
    lvp = np.zeros((NH, LKT_W, 65), f)
    lvp[:, B : B + T, :64] = v
    lvp[:, :, 64] = 1.0
    lvp *= em_l[:, :, None]
    lvp = np.ascontiguousarray(
        lvp.reshape(NH, LV_C, 128, 65).transpose(0, 2, 1, 3)
    ).reshape(NH, 128, LV_C * 65).astype(bf)

    SVP_W = 96 + SV_C * 128
    em_s = np.zeros((NH, SVP_W), f)
    em_s[:, 160 : 160 + TSP] = np.exp(sm)
    sv_pad = np.zeros((NH, SVP_W, 65), f)
    sv_pad[:, 160 : 160 + TSP, :64] = svv
    sv_pad[:, :, 64] = 1.0
    sv_pad *= em_s[:, :, None]
    svp = np.empty((NH, 4, 128, SV_C, 65), f)
    for p in range(4):
        svp[:, p] = (
            sv_pad[:, 32 * p : 32 * p + SV_C * 128]
            .reshape(NH, SV_C, 128, 65)
            .transpose(0, 2, 1, 3)
        )
    # [NH, 4, 128, SV_C, 65] -> [NH, 128, SV_C, 4, 65] (chunk-major so
    # chunk-range load pieces are contiguous per partition)
    svp = np.ascontiguousarray(svp.transpose(0, 2, 3, 1, 4)).astype(bf)

    gvp = np.zeros((NH, 128, 65), f)
    gvp[:, :G, :64] = gvv
    gvp[:, :G, 64] = 1.0
    gvp[:, :G] *= np.exp(gm)[:, :, None]
    gvp = gvp.astype(bf)
    gvj = np.ascontiguousarray(
        gvp.reshape(NCORES, SL, 128, 65).transpose(0, 2, 1, 3)
    ).reshape(NCORES, 128, SL * 65)

    return [
        {
            "qt": qt[c * SL : (c + 1) * SL],
            "lkt": lkt[c * SL : (c + 1) * SL],
            "skt": skt[c * SL : (c + 1) * SL],
            "gkt": gktj[c],
            "lv": lvp[c * SL : (c + 1) * SL],
            "sv": svp[c * SL : (c + 1) * SL],
            "gv": gvj[c],
        }
        for c in range(NCORES)
    ]


_NC_CACHE = {}
LAST_RESULTS = None


def kernel(**inputs):
    global LAST_RESULTS
    if "nc" not in _NC_CACHE:
        _NC_CACHE["nc"] = _build_bass()
    nc = _NC_CACHE["nc"]
    in_maps = _prepare(inputs)
    res = run_bass_kernel_spmd(nc, in_maps, core_ids=list(range(NCORES)))
    LAST_RESULTS = res
    out = np.empty((NH, T, D), np.float32)
    for c in range(NCORES):
        # o is [SL, B, NB, D] -> [SL, NB, B, D] -> [SL, T, D]
        ot = res.results[c]["o"]
        out[c * SL : (c + 1) * SL] = ot.transpose(0, 2, 1, 3).reshape(SL, T, D)
    return out.reshape(N, H, T, D)


# revision 37
# speedup vs baseline: 1.1407x; 1.1407x over previous
"""Block-local sparse attention (LSG-style) on 8 TRN2 NeuronCores.

Sharding: the 32 (n, h) pairs are split 4-per-core (data/head parallel, no
collectives). Host-side numpy prep re-lays-out the inputs so the device
kernel needs no transposes, all bf16:

  - qt : Q^T per head, zero-padded to [128, T]
  - lkt/skt/gkt: local/sparse/global K^T, token-padded with zeros and
    row-padded to 128 partitions.  Every matmul in the kernel then uses the
    same 128-row PE tile shape; the padded stationary rows are zero so the
    junk moving rows contribute nothing.  Uniform tile shapes let the PE
    preload every stationary during the previous matmul (no reload stall at
    64<->128 switches) and keep the PE continuously busy, which lets the
    hardware ramp the tensor-engine clock from 1.2 to 2.4 GHz.
  - lv/sv/gv: V with a ones column appended (col 64), chunked [128, c, 65],
    every row scaled by exp(mask): softmax(QK/8 + m) @ V is computed as
    sum_t exp(s_t) e^{m_t} [V_t, 1] followed by a divide by the accumulated
    last column, so no mask row and no max-subtraction are needed.
    sv holds 4 phase-shifted copies so the 32-token-granular sparse windows
    always start at partition 0 (stored chunk-major so chunk-range loads
    are one descriptor per partition).

The device processes query-block PAIRS: 9 score matmuls per pair (the
shared global chunk and the two shared local chunks stream both blocks'
256 q columns in one matmul) into a 3-bank PSUM region [128, 1536], one
exp(S/8) on ACT into bf16 pp, then 12 PV matmuls (6 per block, N=65)
accumulating [q, V|Z], and a reciprocal-normalize on DVE.  ACT (~1.3us per
pair) and the PE (~1.0us at full clock) are the joint bottlenecks.

Pipelining:
  - All loads and stores share the single qSPDynamicHW DGE, which executes
    DMA instructions serially, so big input batches delay store
    completions.  Inputs load in 3-4 phases per slot (column prefixes
    covering pairs 0-3 / 4-9 / 10-15), issued between stores of earlier
    slots so no burst exceeds ~2us; compute starts after ~0.5 MB.
  - Phase-completion gates use two alternating semaphores per slot parity,
    and each phase's first DMA waits on the previous same-semaphore
    phase's total: DMA completions are unordered across instructions, so a
    gate value is race-free only if no later increment can cross it.
  - Output of a pair is ONE dma (dram laid out [SL, B, NB, D], transposed
    back on the host); ob/rec are quad-buffered with four store semaphores
    so a store delayed behind an input batch has 8 pair-periods of slack.
  - pp is triple-buffered so exp(p) only waits on PV(p-3).

Known hardware landmine (cost several device wedges): an engine reading a
PSUM bank while another engine concurrently touches the same bank (ACT
read + DVE read, or PE matmul write + DVE read) hard-crashes the device
(NRT_EXEC_UNIT_UNRECOVERABLE).  Offloading part of the exp to the DVE via
the Schraudolph int16/bf16 bit trick works numerically (7.7e-3 end to
end) but requires a PSUM bank partitioning between ACT and DVE that did
not fit the 8-bank budget in time; the DVE therefore only touches the pv
banks (strict parity alternation against the PE).
"""

from contextlib import ExitStack

import numpy as np

import concourse.bass as bass
import concourse.mybir as mybir
from concourse.bass_utils import run_bass_kernel_spmd

N, H, T, D = 2, 16, 4096, 64
B = 128          # query block
NB = T // B      # 32
G = 64           # global tokens
TSP = T // 4     # sparse tokens (1024)
NH = N * H       # 32
NCORES = 8
SL = NH // NCORES  # 4 heads per core
NP = SL * NB // 2  # 64 block-pairs per core
PPS = NB // 2      # 16 pairs per slot

LKT_W = T + 2 * B            # 4352 padded local tokens
SKT_W = TSP + 320            # 1344 padded sparse tokens
LV_C = LKT_W // 128          # 34 local V chunks
SV_C = 11                    # sparse V chunks per phase

F32 = mybir.dt.float32
BF16 = mybir.dt.bfloat16
GE = "sem-ge"

# column layout of the per-pair score/prob tile [128, 1536] (3 PSUM banks;
# regions never cross a 512-col bank boundary).  Cols 0:768 = sparse+global
# (ACT exp part A), 768:1280 = shared local chunks (ACT exp part B),
# 1280:1536 = single-block local chunks, exp'd on DVE via the Schraudolph
# int16/bf16 bit trick (rel err ~2%, verified 7.7e-3 end to end).
C_SP1A, C_SP1B = 0, 128
C_SP2A, C_SP2B = 256, 384
C_G = 512        # 256 wide: q of both blocks
C_LOC1 = 768     # 256 wide: local chunk b+1, both blocks
C_LOC2 = 1024    # 256 wide: local chunk b+2, both blocks
C_LOC0 = 1280    # 128: local chunk b, block A only (DVE exp)
C_LOC3 = 1408    # 128: local chunk b+3, block B only (DVE exp)

# Schraudolph constants: bf16 bits of exp(x*0.125) ~= int16(x*SCH_A + SCH_B)
SCH_A = float(128 * 1.4426950408889634 * 0.125)
SCH_B = 16256.0 - 0.057 * 128.0

# per-slot load phases: each phase covers the column needs of a pair range
# (from the access patterns:
#   scores pair p: qt < 256(p+1), lkt < 256p+512, skt < 64p+384
#   PV pair p: lv chunks <= 2p+5, sv chunks <= p//2+2)
# phase = (qt-range, lkt-range, skt-range, lv-chunk-range, sv-chunk-range,
#          gate_hb)  — gate_hb is the first pair needing it.
# Slot 0 uses 4 finer phases so compute starts after ~0.45 MB.
PHASES_S0 = (
    ((0, 512), (0, 768), (0, 448), (0, 8), (0, 3), 0),
    ((512, 1024), (768, 1280), (448, 576), (8, 12), (3, 4), 2),
    ((1024, 2560), (1280, 2816), (576, 960), (12, 24), (4, 7), 4),
    ((2560, 4096), (2816, LKT_W), (960, SKT_W), (24, LV_C), (7, SV_C), 10),
)
PHASES = (
    ((0, 1024), (0, 1280), (0, 576), (0, 12), (0, 5), 0),
    ((1024, 2560), (1280, 2816), (576, 960), (12, 24), (5, 7), 4),
    ((2560, 4096), (2816, LKT_W), (960, SKT_W), (24, LV_C), (7, SV_C), 10),
)
def _slot_phases(s):
    return PHASES_S0 if s == 0 else PHASES


def _phase_ndma(ph):
    # qt, lkt, skt, lv, sv: one DMA each (sv is flat 2-D so it never splits)
    return 5


# A semaphore wait is only race-free at the end of a maximal run of
# consecutive instructions updating that semaphore, so consecutive phases
# alternate between two semaphores per slot parity: di[parity][phase_seq%2].
# DI_SEM[(s, k)] = (parity, alt) and DI_GATE[(s, hb)] = (parity, alt, value).
DI_SEM = {}
DI_GATE = {}
for _u in range(2):
    _cum = [0, 0]
    _seq = 0
    for _s in range(_u, SL, 2):
        for _k, _ph in enumerate(_slot_phases(_s)):
            _alt = _seq % 2
            _cum[_alt] += 16 * _phase_ndma(_ph)
            DI_SEM[(_s, _k)] = (_u, _alt)
            DI_GATE[(_s, _ph[5])] = (_u, _alt, _cum[_alt])
            _seq += 1


def _build_bass():
    nc = bass.Bass("TRN2", num_devices=NCORES, debug=False)

    qt = nc.dram_tensor("qt", [SL, 128, T], BF16, kind="ExternalInput")
    lkt = nc.dram_tensor("lkt", [SL, 128, LKT_W], BF16, kind="ExternalInput")
    skt = nc.dram_tensor("skt", [SL, 128, SKT_W], BF16, kind="ExternalInput")
    gkt = nc.dram_tensor("gkt", [128, SL * 128], BF16, kind="ExternalInput")
    lv = nc.dram_tensor("lv", [SL, 128, LV_C * 65], BF16, kind="ExternalInput")
    sv = nc.dram_tensor("sv", [SL, 128, SV_C * 4 * 65], BF16, kind="ExternalInput")
    gv = nc.dram_tensor("gv", [128, SL * 65], BF16, kind="ExternalInput")
    # output laid out [SL, B, NB, D] so one DMA covers a pair ([128, 2, 64]);
    # host transposes back to [SL, T, D]
    o = nc.dram_tensor("o", [SL, B, NB, D], F32, kind="ExternalOutput")

    EXP = mybir.ActivationFunctionType.Exp

    with ExitStack() as es:
        ec = es.enter_context
        # double-buffered inputs (slot parity)
        qt_t = [ec(nc.sbuf_tensor(f"qt_t{i}", [128, T], BF16)) for i in range(2)]
        lkt_t = [ec(nc.sbuf_tensor(f"lkt_t{i}", [128, LKT_W], BF16)) for i in range(2)]
        skt_t = [ec(nc.sbuf_tensor(f"skt_t{i}", [128, SKT_W], BF16)) for i in range(2)]
        lv_t = [ec(nc.sbuf_tensor(f"lv_t{i}", [128, LV_C * 65], BF16)) for i in range(2)]
        sv_t = [ec(nc.sbuf_tensor(f"sv_t{i}", [128, SV_C * 4 * 65], BF16)) for i in range(2)]
        # globals are tiny: all slots resident, loaded once with one DMA each
        gkt_t = ec(nc.sbuf_tensor("gkt_t", [128, SL * 128], BF16))
        gv_t = ec(nc.sbuf_tensor("gv_t", [128, SL * 65], BF16))
        # per-pair working set
        psS = [ec(nc.psum_tensor(f"psS{i}", [128, 1536], F32)) for i in range(2)]  # 3 banks
        pv = [ec(nc.psum_tensor(f"pv{i}", [128, 512], F32)) for i in range(2)]     # 1 bank
        pp = [ec(nc.sbuf_tensor(f"pp{i}", [128, 1536], BF16)) for i in range(3)]
        rec = [ec(nc.sbuf_tensor(f"rec{i}", [128, 2], F32)) for i in range(4)]
        ob = [ec(nc.sbuf_tensor(f"ob{i}", [128, 128], F32)) for i in range(4)]

        di = [[ec(nc.semaphore(f"di{i}{a}")) for a in range(2)] for i in range(2)]  # input loads, (parity, alternation)
        dg = ec(nc.semaphore("dg"))      # global k/v loads
        st = [ec(nc.semaphore(f"st{i}")) for i in range(4)]  # out stores, p%4 (matches ob buffers)
        pe_s = ec(nc.semaphore("pe_s"))  # +3 per pair: score thirds done
        pe_v = ec(nc.semaphore("pe_v"))  # +1 per pair: PV matmuls done
        act = ec(nc.semaphore("act"))    # +2 per pair: ACT exp halves done
        sch = ec(nc.semaphore("sch"))    # +1 per pair: DVE exp done
        dve = ec(nc.semaphore("dve"))    # +1 per pair: normalize done
        block = ec(nc.Block())

        # last waited-on cumulative value per di semaphore: a later phase
        # crossing that value must itself wait on it (race-checker rule),
        # which is free since the previous same-sem phase finished long ago
        chain = {}

        def phase_pieces(sync, s, k, wait=None):
            u = s % 2
            _, alt = DI_SEM[(s, k)]
            (q0, q1), (l0, l1), (s0, s1), (v0, v1), (c0, c1), _ = _slot_phases(s)[k]
            if wait is not None:
                sync.wait_ge(pe_v, wait)
            dmas = [
                (qt_t[u][:, q0:q1], qt[s, :, q0:q1]),
                (lkt_t[u][:, l0:l1], lkt[s, :, l0:l1]),
                (skt_t[u][:, s0:s1], skt[s, :, s0:s1]),
                (lv_t[u][:, v0 * 65 : v1 * 65], lv[s, :, v0 * 65 : v1 * 65]),
                (sv_t[u][:, c0 * 260 : c1 * 260], sv[s, :, c0 * 260 : c1 * 260]),
            ]
            prev = chain.get((u, alt))
            if prev is not None:
                sync.wait_ge(di[u][alt], prev)
            for dst, src in dmas:
                sync.dma_start(dst, src).then_inc(di[u][alt], 16)
            chain[(u, alt)] = DI_GATE[(s, _slot_phases(s)[k][5])][2]

        @block.sync
        def _(sync):
            phase_pieces(sync, 0, 0)
            sync.dma_start(gkt_t[:], gkt[:]).then_inc(dg, 16)
            sync.dma_start(gv_t[:], gv[:]).then_inc(dg, 16)
            for k in range(1, 4):
                phase_pieces(sync, 0, k)
            for p in range(NP):
                s, hb = divmod(p, PPS)
                b = 2 * hb
                u = p % 4
                sync.dma_start(
                    o[s, :, b : b + 2, :], ob[u][:, 0:128]
                ).wait_op(dve, p + 1, GE).then_inc(st[u], 16)
                if s == 0 and hb in (2, 5, 7):
                    # slot 1's phases, spread through slot 0 (buffers fresh)
                    phase_pieces(sync, 1, {2: 0, 5: 1, 7: 2}[hb])
                if hb == 0 and s >= 1 and s + 1 < SL:
                    phase_pieces(sync, s + 1, 2, wait=16 * s)
                if hb == 9 and s + 2 < SL:
                    phase_pieces(sync, s + 2, 0, wait=16 * s + 10)
                if hb == 13 and s + 2 < SL:
                    phase_pieces(sync, s + 2, 1, wait=16 * s + 14)
            for i in range(4):
                sync.wait_ge(st[i], 16 * (NP // 4))

        def emit_scores(p):
            s, hb = divmod(p, PPS)
            b = 2 * hb
            u = p % 2
            su = s % 2
            if (s, hb) in DI_GATE:
                if p == 0:
                    nc.tensor.wait_ge(dg, 32)
                gu, galt, gval = DI_GATE[(s, hb)]
                nc.tensor.wait_ge(di[gu][galt], gval)
            qA = qt_t[su][:, b * B : (b + 1) * B]
            qB = qt_t[su][:, (b + 1) * B : (b + 2) * B]
            qAB = qt_t[su][:, b * B : (b + 2) * B]
            w1a, w2a = 32 * b, 32 * b + 224
            w1b, w2b = w1a + 32, w2a + 32
            mms = (
                (C_SP1A, 128, skt_t[su][:, w1a : w1a + 128], qA),
                (C_SP1B, 128, skt_t[su][:, w1b : w1b + 128], qB),
                (C_SP2A, 128, skt_t[su][:, w2a : w2a + 128], qA),
                (C_SP2B, 128, skt_t[su][:, w2b : w2b + 128], qB),
                (C_G, 256, gkt_t[:, s * 128 : (s + 1) * 128], qAB),
                (C_LOC1, 256, lkt_t[su][:, (b + 1) * B : (b + 2) * B], qAB),
                (C_LOC2, 256, lkt_t[su][:, (b + 2) * B : (b + 3) * B], qAB),
                (C_LOC0, 128, lkt_t[su][:, b * B : (b + 1) * B], qA),
                (C_LOC3, 128, lkt_t[su][:, (b + 3) * B : (b + 4) * B], qB),
            )
            for kk, (col, w, lhsT, rhs) in enumerate(mms):
                mm = nc.tensor.matmul(
                    psS[u][:, col : col + w],
                    lhsT, rhs,
                    start=True, stop=True,
                )
                if kk in (4, 6, 8):
                    mm.then_inc(pe_s, 1)

        def emit_pv(p):
            s, hb = divmod(p, PPS)
            b = 2 * hb
            u = p % 2
            su = s % 2
            j3 = p % 3
            if p >= 2:
                nc.tensor.wait_ge(dve, p - 1)  # pv[u] free
            kk = 0
            for blk in range(2):
                bb = b + blk
                w1, w2 = 32 * bb, 32 * bb + 224
                c1, r1 = divmod(w1, 128)
                c2, r2 = divmod(w2, 128)
                p1, p2 = r1 // 32, r2 // 32
                if blk == 0:
                    # single-block chunk (C_LOC0, DVE-exp'd) last so one
                    # sch wait covers both blocks by program order
                    lhs = (C_SP1A, C_SP2A, C_G, C_LOC1, C_LOC2, C_LOC0)
                    lvs = (bb + 1, bb + 2, bb)
                else:
                    lhs = (C_SP1B, C_SP2B, C_G + 128, C_LOC1 + 128,
                           C_LOC2 + 128, C_LOC3)
                    lvs = (bb, bb + 1, bb + 2)
                rhss = (
                    sv_t[su][:, (c1 * 4 + p1) * 65 : (c1 * 4 + p1) * 65 + 65],
                    sv_t[su][:, (c2 * 4 + p2) * 65 : (c2 * 4 + p2) * 65 + 65],
                    gv_t[:, s * 65 : (s + 1) * 65],
                    lv_t[su][:, lvs[0] * 65 : lvs[0] * 65 + 65],
                    lv_t[su][:, lvs[1] * 65 : lvs[1] * 65 + 65],
                    lv_t[su][:, lvs[2] * 65 : lvs[2] * 65 + 65],
                )
                out = pv[u][:, blk * 128 : blk * 128 + 65]
                for j in range(6):
                    mm = nc.tensor.matmul(
                        out, pp[j3][:, lhs[j] : lhs[j] + 128], rhss[j],
                        start=(j == 0), stop=(j == 5),
                    )
                    if kk == 0:
                        mm.wait_op(act, p + 1, GE)  # pp ready
                    if kk == 11:
                        mm.then_inc(pe_v, 1)
                    kk += 1

        @block.tensor
        def _(tensor):
            emit_scores(0)
            emit_scores(1)
            for p in range(NP):
                emit_pv(p)
                if p + 2 < NP:
                    emit_scores(p + 2)

        @block.scalar
        def _(scalar):
            for p in range(NP):
                u = p % 2
                j3 = p % 3
                if p >= 3:
                    scalar.wait_ge(pe_v, p - 2)  # pp[j3] free: PV of p-3 done
                nc.scalar.activation(
                    pp[j3][:], psS[u][:, 0:1536], EXP, scale=0.125
                ).wait_op(pe_s, 3 * p + 3, GE).then_inc(act, 1)

        def emit_sch(p):
            # exp via Schraudolph: bf16 bits of exp(x/8) ~= int16(x*A + B);
            # pp[p%3] was last read by PV(p-3), and the preceding DVE
            # reciprocal already waited pe_v >= p, so no extra WAR wait.
            nc.vector.tensor_scalar(
                pp[p % 3][:, 1280:1536].bitcast(mybir.dt.int16),
                psS[p % 2][:, 1280:1536],
                SCH_A, SCH_B,
                mybir.AluOpType.mult, mybir.AluOpType.add,
            ).wait_op(pe_s, 3 * p + 3, GE).then_inc(sch, 1)

        @block.vector
        def _(vector):
            for p in range(NP):
                u = p % 4
                if p >= 4:
                    # ob[u]/rec[u] free once store(p-4) completed
                    vector.wait_ge(st[u], 16 * ((p - 4) // 4 + 1))
                nc.vector.reciprocal(rec[u][:, 0:1], pv[p % 2][:, 64:65]).wait_op(
                    pe_v, p + 1, GE
                )
                nc.vector.reciprocal(rec[u][:, 1:2], pv[p % 2][:, 192:193])
                nc.vector.drain()
                nc.vector.tensor_mul(
                    ob[u][:, 0:64], pv[p % 2][:, 0:64],
                    rec[u][:, 0:1].broadcast_to([128, 64]),
                )
                nc.vector.tensor_mul(
                    ob[u][:, 64:128], pv[p % 2][:, 128:192],
                    rec[u][:, 1:2].broadcast_to([128, 64]),
                ).then_inc(dve, 1)

    return nc


def _prepare(inputs):
    import ml_dtypes

    bf = ml_dtypes.bfloat16
    f = np.float32
    q = np.asarray(inputs["query_layer"], f).reshape(NH, T, D)
    k = np.asarray(inputs["key_layer"], f).reshape(NH, T, D)
    v = np.asarray(inputs["value_layer"], f).reshape(NH, T, D)
    sk = np.asarray(inputs["sparse_key"], f).reshape(NH, TSP, D)
    svv = np.asarray(inputs["sparse_value"], f).reshape(NH, TSP, D)
    gk = np.asarray(inputs["global_key"], f).reshape(NH, G, D)
    gvv = np.asarray(inputs["global_value"], f).reshape(NH, G, D)
    am = np.repeat(np.asarray(inputs["attention_mask"], f)[:, 0, 0, :], H, 0)
    sm = np.repeat(np.asarray(inputs["sparse_mask"], f)[:, 0, 0, :], H, 0)
    gm = np.repeat(np.asarray(inputs["global_mask"], f)[:, 0, 0, :], H, 0)

    # all K^T stationaries (and the moving q) are padded to 128 rows with
    # zeros so every matmul uses the same 128-row PE tile shape — avoiding
    # the stationary-reload serialization at 64<->128 shape switches
    qt = np.zeros((NH, 128, T), f)
    qt[:, :64] = q.transpose(0, 2, 1)
    qt = qt.astype(bf)

    lkt = np.zeros((NH, 128, LKT_W), f)
    lkt[:, :64, B : B + T] = k.transpose(0, 2, 1)
    lkt = lkt.astype(bf)

    skt = np.zeros((NH, 128, SKT_W), f)
    skt[:, :64, 160 : 160 + TSP] = sk.transpose(0, 2, 1)
    skt = skt.astype(bf)

    gkt = np.zeros((NH, 128, 128), f)
    gkt[:, :64, :G] = gk.transpose(0, 2, 1)
    gkt = gkt.astype(bf)
    # per-core: [128, SL*128] (slot-minor so one DMA loads all slots)
    gktj = np.ascontiguousarray(
        gkt.reshape(NCORES, SL, 128, 128).transpose(0, 2, 1, 3)
    ).reshape(NCORES, 128, SL * 128)

    # V_aug rows scaled by exp(mask); pad rows are all-zero
    em_l = np.zeros((NH, LKT_W), f)
    em_l[:, B : B + T] = np.exp(am)
    lvp = np.zeros((NH, LKT_W, 65), f)
    lvp[:, B : B + T, :64] = v
    lvp[:, :, 64] = 1.0
    lvp *= em_l[:, :, None]
    lvp = np.ascontiguousarray(
        lvp.reshape(NH, LV_C, 128, 65).transpose(0, 2, 1, 3)
    ).reshape(NH, 128, LV_C * 65).astype(bf)

    SVP_W = 96 + SV_C * 128
    em_s = np.zeros((NH, SVP_W), f)
    em_s[:, 160 : 160 + TSP] = np.exp(sm)
    sv_pad = np.zeros((NH, SVP_W, 65), f)
    sv_pad[:, 160 : 160 + TSP, :64] = svv
    sv_pad[:, :, 64] = 1.0
    sv_pad *= em_s[:, :, None]
    svp = np.empty((NH, 4, 128, SV_C, 65), f)
    for p in range(4):
        svp[:, p] = (
            sv_pad[:, 32 * p : 32 * p + SV_C * 128]
            .reshape(NH, SV_C, 128, 65)
            .transpose(0, 2, 1, 3)
        )
    # [NH, 4, 128, SV_C, 65] -> [NH, 128, SV_C, 4, 65] (chunk-major so
    # chunk-range load pieces are contiguous per partition)
    svp = np.ascontiguousarray(svp.transpose(0, 2, 3, 1, 4)).astype(bf)
    svp = svp.reshape(NH, 128, SV_C * 4 * 65)

    gvp = np.zeros((NH, 128, 65), f)
    gvp[:, :G, :64] = gvv
    gvp[:, :G, 64] = 1.0
    gvp[:, :G] *= np.exp(gm)[:, :, None]
    gvp = gvp.astype(bf)
    gvj = np.ascontiguousarray(
        gvp.reshape(NCORES, SL, 128, 65).transpose(0, 2, 1, 3)
    ).reshape(NCORES, 128, SL * 65)

    return [
        {
            "qt": qt[c * SL : (c + 1) * SL],
            "lkt": lkt[c * SL : (c + 1) * SL],
            "skt": skt[c * SL : (c + 1) * SL],
            "gkt": gktj[c],
            "lv": lvp[c * SL : (c + 1) * SL],
            "sv": svp[c * SL : (c + 1) * SL],
            "gv": gvj[c],
        }
        for c in range(NCORES)
    ]


_NC_CACHE = {}
LAST_RESULTS = None


def kernel(**inputs):
    global LAST_RESULTS
    if "nc" not in _NC_CACHE:
        _NC_CACHE["nc"] = _build_bass()
    nc = _NC_CACHE["nc"]
    in_maps = _prepare(inputs)
    res = run_bass_kernel_spmd(nc, in_maps, core_ids=list(range(NCORES)))
    LAST_RESULTS = res
    out = np.empty((NH, T, D), np.float32)
    for c in range(NCORES):
        # o is [SL, B, NB, D] -> [SL, NB, B, D] -> [SL, T, D]
        ot = res.results[c]["o"]
        out[c * SL : (c + 1) * SL] = ot.transpose(0, 2, 1, 3).reshape(SL, T, D)
    return out.reshape(N, H, T, D)


# revision 38
# speedup vs baseline: 1.2184x; 1.0682x over previous
"""Block-local sparse attention (LSG-style) on 8 TRN2 NeuronCores.

Sharding: the 32 (n, h) pairs are split 4-per-core (data/head parallel, no
collectives). Host-side numpy prep re-lays-out the inputs so the device
kernel needs no transposes, all bf16:

  - qt : Q^T per head, zero-padded to [128, T]
  - lkt/skt/gkt: local/sparse/global K^T, token-padded with zeros and
    row-padded to 128 partitions.  Every matmul in the kernel then uses the
    same 128-row PE tile shape; the padded stationary rows are zero so the
    junk moving rows contribute nothing.  Uniform tile shapes let the PE
    preload every stationary during the previous matmul (no reload stall at
    64<->128 switches) and keep the PE continuously busy, which lets the
    hardware ramp the tensor-engine clock from 1.2 to 2.4 GHz.
  - lv/sv/gv: V with a ones column appended (col 64), chunked [128, c, 65],
    every row scaled by exp(mask): softmax(QK/8 + m) @ V is computed as
    sum_t exp(s_t) e^{m_t} [V_t, 1] followed by a divide by the accumulated
    last column, so no mask row and no max-subtraction are needed.
    sv holds 4 phase-shifted copies so the 32-token-granular sparse windows
    always start at partition 0 (stored chunk-major so chunk-range loads
    are one descriptor per partition).

The device processes query-block PAIRS: 9 score matmuls per pair (the
shared global chunk and the two shared local chunks stream both blocks'
256 q columns in one matmul) into a 3-bank PSUM region [128, 1536], one
exp(S/8) on ACT into bf16 pp, then 12 PV matmuls (6 per block, N=65)
accumulating [q, V|Z], and a reciprocal-normalize on DVE.  ACT (~1.3us per
pair) and the PE (~1.0us at full clock) are the joint bottlenecks.

Pipelining:
  - All loads and stores share the single qSPDynamicHW DGE, which executes
    DMA instructions serially, so big input batches delay store
    completions.  Inputs load in 3-4 phases per slot (column prefixes
    covering pairs 0-3 / 4-9 / 10-15), issued between stores of earlier
    slots so no burst exceeds ~2us; compute starts after ~0.5 MB.
  - Phase-completion gates use two alternating semaphores per slot parity,
    and each phase's first DMA waits on the previous same-semaphore
    phase's total: DMA completions are unordered across instructions, so a
    gate value is race-free only if no later increment can cross it.
  - Output of a pair is ONE dma (dram laid out [SL, B, NB, D], transposed
    back on the host); ob/rec are quad-buffered with four store semaphores
    so a store delayed behind an input batch has 8 pair-periods of slack.
  - pp is triple-buffered so exp(p) only waits on PV(p-3).

Known hardware landmine (cost several device wedges): an engine reading a
PSUM bank while another engine concurrently touches the same bank (ACT
read + DVE read, or PE matmul write + DVE read) hard-crashes the device
(NRT_EXEC_UNIT_UNRECOVERABLE).  Offloading part of the exp to the DVE via
the Schraudolph int16/bf16 bit trick works numerically (7.7e-3 end to
end) but requires a PSUM bank partitioning between ACT and DVE that did
not fit the 8-bank budget in time; the DVE therefore only touches the pv
banks (strict parity alternation against the PE).
"""

from contextlib import ExitStack

import numpy as np

import concourse.bass as bass
import concourse.mybir as mybir
from concourse.bass_utils import run_bass_kernel_spmd

N, H, T, D = 2, 16, 4096, 64
B = 128          # query block
NB = T // B      # 32
G = 64           # global tokens
TSP = T // 4     # sparse tokens (1024)
NH = N * H       # 32
NCORES = 8
SL = NH // NCORES  # 4 heads per core
NP = SL * NB // 2  # 64 block-pairs per core
PPS = NB // 2      # 16 pairs per slot

LKT_W = T + 2 * B            # 4352 padded local tokens
SKT_W = TSP + 320            # 1344 padded sparse tokens
LV_C = LKT_W // 128          # 34 local V chunks
SV_C = 11                    # sparse V chunks per phase

F32 = mybir.dt.float32
BF16 = mybir.dt.bfloat16
GE = "sem-ge"

# column layout of the per-pair score/prob tile [128, 1536] (3 PSUM banks;
# regions never cross a 512-col bank boundary).  Cols 0:768 = sparse+global
# (ACT exp part A), 768:1280 = shared local chunks (ACT exp part B),
# 1280:1536 = single-block local chunks, exp'd on DVE via the Schraudolph
# int16/bf16 bit trick (rel err ~2%, verified 7.7e-3 end to end).
C_SP1A, C_SP1B = 0, 128
C_SP2A, C_SP2B = 256, 384
C_G = 512        # 256 wide: q of both blocks
C_LOC1 = 768     # 256 wide: local chunk b+1, both blocks
C_LOC2 = 1024    # 256 wide: local chunk b+2, both blocks
C_LOC0 = 1280    # 128: local chunk b, block A only (DVE exp)
C_LOC3 = 1408    # 128: local chunk b+3, block B only (DVE exp)

# Schraudolph constants: bf16 bits of exp(x*0.125) ~= int16(x*SCH_A + SCH_B)
SCH_A = float(128 * 1.4426950408889634 * 0.125)
SCH_B = 16256.0 - 0.057 * 128.0

# per-slot load phases: each phase covers the column needs of a pair range
# (from the access patterns:
#   scores pair p: qt < 256(p+1), lkt < 256p+512, skt < 64p+384
#   PV pair p: lv chunks <= 2p+5, sv chunks <= p//2+2)
# phase = (qt-range, lkt-range, skt-range, lv-chunk-range, sv-chunk-range,
#          gate_hb)  — gate_hb is the first pair needing it.
# Slot 0 uses 4 finer phases so compute starts after ~0.45 MB.
PHASES_S0 = (
    ((0, 512), (0, 768), (0, 448), (0, 8), (0, 3), 0),
    ((512, 1024), (768, 1280), (448, 576), (8, 12), (3, 4), 2),
    ((1024, 2560), (1280, 2816), (576, 960), (12, 24), (4, 7), 4),
    ((2560, 4096), (2816, LKT_W), (960, SKT_W), (24, LV_C), (7, SV_C), 10),
)
PHASES = (
    ((0, 1024), (0, 1280), (0, 576), (0, 12), (0, 5), 0),
    ((1024, 2560), (1280, 2816), (576, 960), (12, 24), (5, 7), 4),
    ((2560, 4096), (2816, LKT_W), (960, SKT_W), (24, LV_C), (7, SV_C), 10),
)
def _slot_phases(s):
    return PHASES_S0 if s == 0 else PHASES


def _phase_ndma(ph):
    # qt, lkt, skt, lv, sv: one DMA each (sv is flat 2-D so it never splits)
    return 5


# A semaphore wait is only race-free at the end of a maximal run of
# consecutive instructions updating that semaphore, so consecutive phases
# alternate between two semaphores per slot parity: di[parity][phase_seq%2].
# DI_SEM[(s, k)] = (parity, alt) and DI_GATE[(s, hb)] = (parity, alt, value).
DI_SEM = {}
DI_GATE = {}
for _u in range(2):
    _cum = [0, 0]
    _seq = 0
    for _s in range(_u, SL, 2):
        for _k, _ph in enumerate(_slot_phases(_s)):
            _alt = _seq % 2
            _cum[_alt] += 16 * _phase_ndma(_ph)
            DI_SEM[(_s, _k)] = (_u, _alt)
            DI_GATE[(_s, _ph[5])] = (_u, _alt, _cum[_alt])
            _seq += 1


def _build_bass():
    nc = bass.Bass("TRN2", num_devices=NCORES, debug=False)

    qt = nc.dram_tensor("qt", [SL, 128, T], BF16, kind="ExternalInput")
    lkt = nc.dram_tensor("lkt", [SL, 128, LKT_W], BF16, kind="ExternalInput")
    skt = nc.dram_tensor("skt", [SL, 128, SKT_W], BF16, kind="ExternalInput")
    gkt = nc.dram_tensor("gkt", [128, SL * 128], BF16, kind="ExternalInput")
    lv = nc.dram_tensor("lv", [SL, 128, LV_C * 65], BF16, kind="ExternalInput")
    sv = nc.dram_tensor("sv", [SL, 128, SV_C * 4 * 65], BF16, kind="ExternalInput")
    gv = nc.dram_tensor("gv", [128, SL * 65], BF16, kind="ExternalInput")
    # output laid out [SL, B, NB, D] so one DMA covers a pair ([128, 2, 64]);
    # host transposes back to [SL, T, D]
    o = nc.dram_tensor("o", [SL, B, NB, D], F32, kind="ExternalOutput")

    EXP = mybir.ActivationFunctionType.Exp

    with ExitStack() as es:
        ec = es.enter_context
        # double-buffered inputs (slot parity)
        qt_t = [ec(nc.sbuf_tensor(f"qt_t{i}", [128, T], BF16)) for i in range(2)]
        lkt_t = [ec(nc.sbuf_tensor(f"lkt_t{i}", [128, LKT_W], BF16)) for i in range(2)]
        skt_t = [ec(nc.sbuf_tensor(f"skt_t{i}", [128, SKT_W], BF16)) for i in range(2)]
        lv_t = [ec(nc.sbuf_tensor(f"lv_t{i}", [128, LV_C * 65], BF16)) for i in range(2)]
        sv_t = [ec(nc.sbuf_tensor(f"sv_t{i}", [128, SV_C * 4 * 65], BF16)) for i in range(2)]
        # globals are tiny: all slots resident, loaded once with one DMA each
        gkt_t = ec(nc.sbuf_tensor("gkt_t", [128, SL * 128], BF16))
        gv_t = ec(nc.sbuf_tensor("gv_t", [128, SL * 65], BF16))
        # per-pair working set
        psS = [ec(nc.psum_tensor(f"psS{i}", [128, 1536], F32)) for i in range(2)]  # 3 banks
        pv = [ec(nc.psum_tensor(f"pv{i}", [128, 512], F32)) for i in range(2)]     # 1 bank
        pp = [ec(nc.sbuf_tensor(f"pp{i}", [128, 1536], BF16)) for i in range(3)]
        rec = [ec(nc.sbuf_tensor(f"rec{i}", [128, 2], F32)) for i in range(4)]
        ob = [ec(nc.sbuf_tensor(f"ob{i}", [128, 128], F32)) for i in range(4)]

        di = [[ec(nc.semaphore(f"di{i}{a}")) for a in range(2)] for i in range(2)]  # input loads, (parity, alternation)
        dg = ec(nc.semaphore("dg"))      # global k/v loads
        st = [ec(nc.semaphore(f"st{i}")) for i in range(4)]  # out stores, p%4 (matches ob buffers)
        pe_s = ec(nc.semaphore("pe_s"))  # +3 per pair: score thirds done
        pe_v = ec(nc.semaphore("pe_v"))  # +1 per pair: PV matmuls done
        act = ec(nc.semaphore("act"))    # +2 per pair: ACT exp halves done
        sch = ec(nc.semaphore("sch"))    # +1 per pair: DVE exp done
        dve = ec(nc.semaphore("dve"))    # +1 per pair: normalize done
        block = ec(nc.Block())

        # last waited-on cumulative value per di semaphore: a later phase
        # crossing that value must itself wait on it (race-checker rule),
        # which is free since the previous same-sem phase finished long ago
        chain = {}

        def phase_pieces(sync, s, k, wait=None):
            u = s % 2
            _, alt = DI_SEM[(s, k)]
            (q0, q1), (l0, l1), (s0, s1), (v0, v1), (c0, c1), _ = _slot_phases(s)[k]
            if wait is not None:
                sync.wait_ge(pe_v, wait)
            dmas = [
                (qt_t[u][:, q0:q1], qt[s, :, q0:q1]),
                (lkt_t[u][:, l0:l1], lkt[s, :, l0:l1]),
                (skt_t[u][:, s0:s1], skt[s, :, s0:s1]),
                (lv_t[u][:, v0 * 65 : v1 * 65], lv[s, :, v0 * 65 : v1 * 65]),
                (sv_t[u][:, c0 * 260 : c1 * 260], sv[s, :, c0 * 260 : c1 * 260]),
            ]
            prev = chain.get((u, alt))
            if prev is not None:
                sync.wait_ge(di[u][alt], prev)
            for dst, src in dmas:
                sync.dma_start(dst, src).then_inc(di[u][alt], 16)
            chain[(u, alt)] = DI_GATE[(s, _slot_phases(s)[k][5])][2]

        @block.sync
        def _(sync):
            phase_pieces(sync, 0, 0)
            sync.dma_start(gkt_t[:], gkt[:]).then_inc(dg, 16)
            sync.dma_start(gv_t[:], gv[:]).then_inc(dg, 16)
            for k in range(1, 4):
                phase_pieces(sync, 0, k)
            for p in range(NP):
                s, hb = divmod(p, PPS)
                b = 2 * hb
                u = p % 4
                sync.dma_start(
                    o[s, :, b : b + 2, :], ob[u][:, 0:128]
                ).wait_op(dve, p + 1, GE).then_inc(st[u], 16)
                if s == 0 and hb in (2, 5, 7):
                    # slot 1's phases, spread through slot 0 (buffers fresh)
                    phase_pieces(sync, 1, {2: 0, 5: 1, 7: 2}[hb])
                if hb == 0 and s >= 1 and s + 1 < SL:
                    phase_pieces(sync, s + 1, 2, wait=16 * s)
                if hb == 9 and s + 2 < SL:
                    phase_pieces(sync, s + 2, 0, wait=16 * s + 10)
                if hb == 13 and s + 2 < SL:
                    phase_pieces(sync, s + 2, 1, wait=16 * s + 14)
            for i in range(4):
                sync.wait_ge(st[i], 16 * (NP // 4))

        def emit_scores(p):
            s, hb = divmod(p, PPS)
            b = 2 * hb
            u = p % 2
            su = s % 2
            if (s, hb) in DI_GATE:
                if p == 0:
                    nc.tensor.wait_ge(dg, 32)
                gu, galt, gval = DI_GATE[(s, hb)]
                nc.tensor.wait_ge(di[gu][galt], gval)
            qA = qt_t[su][:, b * B : (b + 1) * B]
            qB = qt_t[su][:, (b + 1) * B : (b + 2) * B]
            qAB = qt_t[su][:, b * B : (b + 2) * B]
            w1a, w2a = 32 * b, 32 * b + 224
            w1b, w2b = w1a + 32, w2a + 32
            mms = (
                (C_SP1A, 128, skt_t[su][:, w1a : w1a + 128], qA),
                (C_SP1B, 128, skt_t[su][:, w1b : w1b + 128], qB),
                (C_SP2A, 128, skt_t[su][:, w2a : w2a + 128], qA),
                (C_SP2B, 128, skt_t[su][:, w2b : w2b + 128], qB),
                (C_G, 256, gkt_t[:, s * 128 : (s + 1) * 128], qAB),
                (C_LOC1, 256, lkt_t[su][:, (b + 1) * B : (b + 2) * B], qAB),
                (C_LOC2, 256, lkt_t[su][:, (b + 2) * B : (b + 3) * B], qAB),
                (C_LOC0, 128, lkt_t[su][:, b * B : (b + 1) * B], qA),
                (C_LOC3, 128, lkt_t[su][:, (b + 3) * B : (b + 4) * B], qB),
            )
            for kk, (col, w, lhsT, rhs) in enumerate(mms):
                mm = nc.tensor.matmul(
                    psS[u][:, col : col + w],
                    lhsT, rhs,
                    start=True, stop=True,
                )
                if kk in (4, 6, 8):
                    mm.then_inc(pe_s, 1)

        def emit_pv(p):
            s, hb = divmod(p, PPS)
            b = 2 * hb
            u = p % 2
            su = s % 2
            j3 = p % 3
            if p >= 2:
                nc.tensor.wait_ge(dve, p - 1)  # pv[u] free
            kk = 0
            for blk in range(2):
                bb = b + blk
                w1, w2 = 32 * bb, 32 * bb + 224
                c1, r1 = divmod(w1, 128)
                c2, r2 = divmod(w2, 128)
                p1, p2 = r1 // 32, r2 // 32
                if blk == 0:
                    # single-block chunk (C_LOC0, DVE-exp'd) last so one
                    # sch wait covers both blocks by program order
                    lhs = (C_SP1A, C_SP2A, C_G, C_LOC1, C_LOC2, C_LOC0)
                    lvs = (bb + 1, bb + 2, bb)
                else:
                    lhs = (C_SP1B, C_SP2B, C_G + 128, C_LOC1 + 128,
                           C_LOC2 + 128, C_LOC3)
                    lvs = (bb, bb + 1, bb + 2)
                rhss = (
                    sv_t[su][:, (c1 * 4 + p1) * 65 : (c1 * 4 + p1) * 65 + 65],
                    sv_t[su][:, (c2 * 4 + p2) * 65 : (c2 * 4 + p2) * 65 + 65],
                    gv_t[:, s * 65 : (s + 1) * 65],
                    lv_t[su][:, lvs[0] * 65 : lvs[0] * 65 + 65],
                    lv_t[su][:, lvs[1] * 65 : lvs[1] * 65 + 65],
                    lv_t[su][:, lvs[2] * 65 : lvs[2] * 65 + 65],
                )
                out = pv[u][:, blk * 128 : blk * 128 + 65]
                for j in range(6):
                    mm = nc.tensor.matmul(
                        out, pp[j3][:, lhs[j] : lhs[j] + 128], rhss[j],
                        start=(j == 0), stop=(j == 5),
                    )
                    if kk == 0:
                        mm.wait_op(act, 2 * p + 1, GE)  # pp sp+G ready
                    elif kk == 3:
                        mm.wait_op(act, 2 * p + 2, GE)  # pp locals ready
                    if kk == 11:
                        mm.then_inc(pe_v, 1)
                    kk += 1

        @block.tensor
        def _(tensor):
            emit_scores(0)
            emit_scores(1)
            for p in range(NP):
                emit_pv(p)
                if p + 2 < NP:
                    emit_scores(p + 2)

        @block.scalar
        def _(scalar):
            for p in range(NP):
                u = p % 2
                j3 = p % 3
                if p >= 3:
                    scalar.wait_ge(pe_v, p - 2)  # pp[j3] free: PV of p-3 done
                nc.scalar.activation(
                    pp[j3][:, 0:768], psS[u][:, 0:768], EXP, scale=0.125
                ).wait_op(pe_s, 3 * p + 1, GE).then_inc(act, 1)
                nc.scalar.activation(
                    pp[j3][:, 768:1536], psS[u][:, 768:1536], EXP, scale=0.125
                ).wait_op(pe_s, 3 * p + 3, GE).then_inc(act, 1)

        def emit_sch(p):
            # exp via Schraudolph: bf16 bits of exp(x/8) ~= int16(x*A + B);
            # pp[p%3] was last read by PV(p-3), and the preceding DVE
            # reciprocal already waited pe_v >= p, so no extra WAR wait.
            nc.vector.tensor_scalar(
                pp[p % 3][:, 1280:1536].bitcast(mybir.dt.int16),
                psS[p % 2][:, 1280:1536],
                SCH_A, SCH_B,
                mybir.AluOpType.mult, mybir.AluOpType.add,
            ).wait_op(pe_s, 3 * p + 3, GE).then_inc(sch, 1)

        @block.vector
        def _(vector):
            for p in range(NP):
                u = p % 4
                if p >= 4:
                    # ob[u]/rec[u] free once store(p-4) completed
                    vector.wait_ge(st[u], 16 * ((p - 4) // 4 + 1))
                nc.vector.reciprocal(rec[u][:, 0:1], pv[p % 2][:, 64:65]).wait_op(
                    pe_v, p + 1, GE
                )
                nc.vector.reciprocal(rec[u][:, 1:2], pv[p % 2][:, 192:193])
                nc.vector.drain()
                nc.vector.tensor_mul(
                    ob[u][:, 0:64], pv[p % 2][:, 0:64],
                    rec[u][:, 0:1].broadcast_to([128, 64]),
                )
                nc.vector.tensor_mul(
                    ob[u][:, 64:128], pv[p % 2][:, 128:192],
                    rec[u][:, 1:2].broadcast_to([128, 64]),
                ).then_inc(dve, 1)

    return nc


def _prepare(inputs):
    import ml_dtypes

    bf = ml_dtypes.bfloat16
    f = np.float32
    q = np.asarray(inputs["query_layer"], f).reshape(NH, T, D)
    k = np.asarray(inputs["key_layer"], f).reshape(NH, T, D)
    v = np.asarray(inputs["value_layer"], f).reshape(NH, T, D)
    sk = np.asarray(inputs["sparse_key"], f).reshape(NH, TSP, D)
    svv = np.asarray(inputs["sparse_value"], f).reshape(NH, TSP, D)
    gk = np.asarray(inputs["global_key"], f).reshape(NH, G, D)
    gvv = np.asarray(inputs["global_value"], f).reshape(NH, G, D)
    am = np.repeat(np.asarray(inputs["attention_mask"], f)[:, 0, 0, :], H, 0)
    sm = np.repeat(np.asarray(inputs["sparse_mask"], f)[:, 0, 0, :], H, 0)
    gm = np.repeat(np.asarray(inputs["global_mask"], f)[:, 0, 0, :], H, 0)

    # all K^T stationaries (and the moving q) are padded to 128 rows with
    # zeros so every matmul uses the same 128-row PE tile shape — avoiding
    # the stationary-reload serialization at 64<->128 shape switches
    qt = np.zeros((NH, 128, T), f)
    qt[:, :64] = q.transpose(0, 2, 1)
    qt = qt.astype(bf)

    lkt = np.zeros((NH, 128, LKT_W), f)
    lkt[:, :64, B : B + T] = k.transpose(0, 2, 1)
    lkt = lkt.astype(bf)

    skt = np.zeros((NH, 128, SKT_W), f)
    skt[:, :64, 160 : 160 + TSP] = sk.transpose(0, 2, 1)
    skt = skt.astype(bf)

    gkt = np.zeros((NH, 128, 128), f)
    gkt[:, :64, :G] = gk.transpose(0, 2, 1)
    gkt = gkt.astype(bf)
    # per-core: [128, SL*128] (slot-minor so one DMA loads all slots)
    gktj = np.ascontiguousarray(
        gkt.reshape(NCORES, SL, 128, 128).transpose(0, 2, 1, 3)
    ).reshape(NCORES, 128, SL * 128)

    # V_aug rows scaled by exp(mask); pad rows are all-zero
    em_l = np.zeros((NH, LKT_W), f)
    em_l[:, B : B + T] = np.exp(am)
    lvp = np.zeros((NH, LKT_W, 65), f)
    lvp[:, B : B + T, :64] = v
    lvp[:, :, 64] = 1.0
    lvp *= em_l[:, :, None]
    lvp = np.ascontiguousarray(
        lvp.reshape(NH, LV_C, 128, 65).transpose(0, 2, 1, 3)
    ).reshape(NH, 128, LV_C * 65).astype(bf)

    SVP_W = 96 + SV_C * 128
    em_s = np.zeros((NH, SVP_W), f)
    em_s[:, 160 : 160 + TSP] = np.exp(sm)
    sv_pad = np.zeros((NH, SVP_W, 65), f)
    sv_pad[:, 160 : 160 + TSP, :64] = svv
    sv_pad[:, :, 64] = 1.0
    sv_pad *= em_s[:, :, None]
    svp = np.empty((NH, 4, 128, SV_C, 65), f)
    for p in range(4):
        svp[:, p] = (
            sv_pad[:, 32 * p : 32 * p + SV_C * 128]
            .reshape(NH, SV_C, 128, 65)
            .transpose(0, 2, 1, 3)
        )
    # [NH, 4, 128, SV_C, 65] -> [NH, 128, SV_C, 4, 65] (chunk-major so
    # chunk-range load pieces are contiguous per partition)
    svp = np.ascontiguousarray(svp.transpose(0, 2, 3, 1, 4)).astype(bf)
    svp = svp.reshape(NH, 128, SV_C * 4 * 65)

    gvp = np.zeros((NH, 128, 65), f)
    gvp[:, :G, :64] = gvv
    gvp[:, :G, 64] = 1.0
    gvp[:, :G] *= np.exp(gm)[:, :, None]
    gvp = gvp.astype(bf)
    gvj = np.ascontiguousarray(
        gvp.reshape(NCORES, SL, 128, 65).transpose(0, 2, 1, 3)
    ).reshape(NCORES, 128, SL * 65)

    return [
        {
            "qt": qt[c * SL : (c + 1) * SL],
            "lkt": lkt[c * SL : (c + 1) * SL],
            "skt": skt[c * SL : (c + 1) * SL],
            "gkt": gktj[c],
            "lv": lvp[c * SL : (c + 1) * SL],
            "sv": svp[c * SL : (c + 1) * SL],
            "gv": gvj[c],
        }
        for c in range(NCORES)
    ]


_NC_CACHE = {}
LAST_RESULTS = None


def kernel(**inputs):
    global LAST_RESULTS
    if "nc" not in _NC_CACHE:
        _NC_CACHE["nc"] = _build_bass()
    nc = _NC_CACHE["nc"]
    in_maps = _prepare(inputs)
    res = run_bass_kernel_spmd(nc, in_maps, core_ids=list(range(NCORES)))
    LAST_RESULTS = res
    out = np.empty((NH, T, D), np.float32)
    for c in range(NCORES):
        # o is [SL, B, NB, D] -> [SL, NB, B, D] -> [SL, T, D]
        ot = res.results[c]["o"]
        out[c * SL : (c + 1) * SL] = ot.transpose(0, 2, 1, 3).reshape(SL, T, D)
    return out.reshape(N, H, T, D)
